# revision 3
# baseline (speedup 1.0000x reference)
"""GCN (2-layer + MLP head) on 8 Trainium2 NeuronCores — v3.

Aggregation = HWDGE indirect-DMA gather (token-major, dst-sorted) + one-hot
matmul scatter on the TensorEngine, fused per dst-tile with the pointwise and
the next layer's GEMM. No SWDGE descriptors, no Q7 per-token work.

Per core (12500 dst nodes, ~400k in-edges):
  L1 GEMM (bf16)  -> slice1 [12544, 64] bf16 -> AllGather -> table1 [100352, 64]
  per dst-tile m (98 of 128 nodes):
    indirect gather of the tile's tokens (dst-sorted, padded to 128-chunks)
    one-hot matmuls accumulate sum_{src} h[src] in PSUM (+ identity matmul
    adds the self-loop slice term)
    pointwise (x dinv_dst + bias, relu) -> h tile -> PE transpose -> L2 GEMM
  ... same for layer 2, then the MLP head.

dinv_src is folded into x on the host; dinv_dst applied per-tile (partition
scalar). Tokens within a tile are in arbitrary order (the one-hot absorbs the
permutation); pad slots use offset 0 with dstv=-1 (one-hot row = 0).
"""
import numpy as np
import ml_dtypes

import concourse.bacc as bacc
import concourse.bass as bass
import concourse.mybir as mybir
from concourse.tile import TileContext
from concourse.bass_utils import run_bass_kernel_spmd

N = 100000
NS_RAW = 12500
NS = 12544            # padded nodes per core (98 * 128)
NT = NS // 128        # 98 tiles
N8 = NS * 8           # 100352 table rows
IN_CH, HID, HID2, OUT = 256, 64, 32, 2

_compiled = {}


def _build_schedule(src, dst):
    """Per-core dst-sorted token chunks.

    Returns (offs [8,128,TOTCH] int32, dstv [8,128,TOTCH] bf16, n_m [98]).
    Token t of chunk j in tile m sits at offs[c][t%128, base_m+j]; its gather
    row is the global table index of its src; dstv holds the tile-local dst
    (or -1 for pads).
    """
    tbl = (src // NS_RAW) * NS + (src % NS_RAW)
    core = dst // NS_RAW
    dl_all = dst % NS_RAW

    per_core = []
    counts = np.zeros((8, NT), np.int64)
    for c in range(8):
        m = core == c
        s = tbl[m]
        d = dl_all[m]
        o = np.argsort(d, kind="stable")
        s, d = s[o], d[o]
        counts[c] = np.bincount(d // 128, minlength=NT)
        per_core.append((s, d))
    n_m = (np.ceil(counts.max(axis=0) / 128.0).astype(np.int64)).clip(min=1)
    base = np.concatenate([[0], np.cumsum(n_m)])
    TOTCH = int(base[-1])

    offs = np.zeros((8, 128, TOTCH), np.int32)
    dstv = np.full((8, 128, TOTCH), -1.0, ml_dtypes.bfloat16)
    for c in range(8):
        s, d = per_core[c]
        t0 = 0
        for mt in range(NT):
            n = int(counts[c, mt])
            seg_s = s[t0:t0 + n]
            seg_d = d[t0:t0 + n] - 128 * mt
            t0 += n
            slots = int(n_m[mt]) * 128
            ps = np.zeros(slots, np.int32)
            pd = np.full(slots, -1.0, np.float32)
            ps[:n] = seg_s
            pd[:n] = seg_d
            offs[c, :, base[mt]:base[mt + 1]] = ps.reshape(-1, 128).T
            dstv[c, :, base[mt]:base[mt + 1]] = pd.reshape(-1, 128).T.astype(ml_dtypes.bfloat16)
    return offs, dstv, [int(x) for x in n_m]


def _build_program(n_m):
    nc = bacc.Bacc(None, target_bir_lowering=False)
    dt = mybir.dt
    P = nc.declare_dram_parameter
    TOTCH = sum(n_m)
    base = np.concatenate([[0], np.cumsum(n_m)]).astype(int)

    xT = P("xT", [IN_CH, NS], dt.bfloat16, isOutput=False)
    w1p = P("w1p", [128, 128], dt.bfloat16, isOutput=False)
    w2 = P("w2", [HID, HID], dt.bfloat16, isOutput=False)
    wh1 = P("wh1", [HID, HID2], dt.bfloat16, isOutput=False)
    wh2 = P("wh2", [HID2, OUT], dt.bfloat16, isOutput=False)
    b1f = P("b1f", [128, HID], dt.float32, isOutput=False)
    b2f = P("b2f", [128, HID], dt.float32, isOutput=False)
    bh1 = P("bh1", [HID2, 1], dt.float32, isOutput=False)
    bh2 = P("bh2", [OUT, 1], dt.float32, isOutput=False)
    dinvP = P("dinvP", [128, NT], dt.float32, isOutput=False)
    iotaP = P("iotaP", [128, 128], dt.bfloat16, isOutput=False)
    identP = P("identP", [128, 128], dt.bfloat16, isOutput=False)
    offsP = P("offsP", [128, TOTCH], dt.int32, isOutput=False)
    dstvP = P("dstvP", [128, TOTCH], dt.bfloat16, isOutput=False)
    outT = P("outT", [OUT, NS], dt.float32, isOutput=True)

    slice_d = [nc.dram_tensor(f"slice{l}", [NS, HID], dt.bfloat16) for l in (1, 2)]
    table_d = [nc.dram_tensor(f"table{l}", [N8, HID], dt.bfloat16, addr_space="Shared")
               for l in (1, 2)]

    relu = mybir.ActivationFunctionType.Relu
    mult = mybir.AluOpType.mult
    add = mybir.AluOpType.add
    iseq = mybir.AluOpType.is_equal

    with TileContext(nc) as tc:
        with tc.tile_pool(name="const", bufs=1) as cp, \
             tc.tile_pool(name="gem", bufs=3) as gp, \
             tc.tile_pool(name="agg", bufs=8) as ap, \
             tc.tile_pool(name="pw", bufs=4) as wp, \
             tc.tile_pool(name="ps", bufs=3, space="PSUM") as pp, \
             tc.tile_pool(name="ps2", bufs=1, space="PSUM") as pp2:
            w1sb = cp.tile([128, 128], dt.bfloat16)
            nc.sync.dma_start(out=w1sb[:], in_=w1p[:])
            w2sb = cp.tile([HID, HID], dt.bfloat16)
            nc.sync.dma_start(out=w2sb[:], in_=w2[:])
            wh1sb = cp.tile([HID, HID2], dt.bfloat16)
            nc.sync.dma_start(out=wh1sb[:], in_=wh1[:])
            wh2sb = cp.tile([HID2, OUT], dt.bfloat16)
            nc.sync.dma_start(out=wh2sb[:], in_=wh2[:])
            b1sb = cp.tile([128, HID], dt.float32)
            nc.sync.dma_start(out=b1sb[:], in_=b1f[:])
            b2sb = cp.tile([128, HID], dt.float32)
            nc.sync.dma_start(out=b2sb[:], in_=b2f[:])
            bh1sb = cp.tile([HID2, 1], dt.float32)
            nc.sync.dma_start(out=bh1sb[:], in_=bh1[:])
            bh2sb = cp.tile([OUT, 1], dt.float32)
            nc.sync.dma_start(out=bh2sb[:], in_=bh2[:])
            dsb = cp.tile([128, NT], dt.float32)
            nc.sync.dma_start(out=dsb[:], in_=dinvP[:])
            iota = cp.tile([128, 128], dt.bfloat16)
            nc.sync.dma_start(out=iota[:], in_=iotaP[:])
            ident = cp.tile([128, 128], dt.bfloat16)
            nc.sync.dma_start(out=ident[:], in_=identP[:])
            offs = cp.tile([128, TOTCH], dt.int32)
            nc.sync.dma_start(out=offs[:], in_=offsP[:])
            dstv = cp.tile([128, TOTCH], dt.bfloat16)
            nc.sync.dma_start(out=dstv[:], in_=dstvP[:])

            # ---- layer-1 GEMM ----
            for m in range(NT):
                mc = slice(m * 128, (m + 1) * 128)
                xa = gp.tile([128, 128], dt.bfloat16, tag="xa")
                nc.sync.dma_start(out=xa[:], in_=xT[0:128, mc])
                xb = gp.tile([128, 128], dt.bfloat16, tag="xb")
                nc.sync.dma_start(out=xb[:], in_=xT[128:256, mc])
                ps = pp.tile([128, HID], dt.float32, tag="acc")
                nc.tensor.matmul(ps[:], xa[:], w1sb[:, 0:HID], start=True, stop=False)
                nc.tensor.matmul(ps[:], xb[:], w1sb[:, HID:128], start=False, stop=True)
                hw = gp.tile([128, HID], dt.bfloat16, tag="hw")
                nc.vector.tensor_copy(hw[:], ps[:])
                nc.sync.dma_start(out=slice_d[0][mc, :], in_=hw[:])

            def agg_layer(layer):
                """Fused aggregation + pointwise (+ next-stage GEMM / head)."""
                table = table_d[layer]
                slc = slice_d[layer]
                bsb = b1sb if layer == 0 else b2sb
                nc.gpsimd.collective_compute(
                    "AllGather", mybir.AluOpType.bypass,
                    replica_groups=[list(range(8))],
                    ins=[slc[:]], outs=[table[:]])
                for m in range(NT):
                    mc = slice(m * 128, (m + 1) * 128)
                    nm = n_m[m]
                    b0 = int(base[m])
                    # one-hot block for all nm chunks in one DVE op
                    oh = ap.tile([128, nm * 128], dt.bfloat16, tag="oh")
                    oh3 = oh[:].rearrange("p (n q) -> p n q", q=128)
                    nc.vector.tensor_tensor(
                        out=oh3,
                        in0=iota[:].rearrange("p (n q) -> p n q", n=1).to_broadcast([128, nm, 128]),
                        in1=dstv[:, b0:b0 + nm].rearrange("p (n q) -> p n q", q=1).to_broadcast([128, nm, 128]),
                        op=iseq)
                    st = wp.tile([128, HID], dt.bfloat16, tag="st")
                    nc.sync.dma_start(out=st[:], in_=slc[mc, :])
                    psA = pp.tile([128, HID], dt.float32, tag="acc")
                    nc.tensor.matmul(psA[:], ident[:], st[:], start=True, stop=False)
                    for j in range(nm):
                        g = ap.tile([128, HID], dt.bfloat16, tag="g")
                        nc.gpsimd.indirect_dma_start(
                            out=g[:], out_offset=None, in_=table[:],
                            in_offset=bass.IndirectOffsetOnAxis(
                                ap=offs[:, b0 + j:b0 + j + 1], axis=0))
                        nc.tensor.matmul(
                            psA[:], oh[:, j * 128:(j + 1) * 128], g[:],
                            start=False, stop=(j == nm - 1))
                    s = wp.tile([128, HID], dt.bfloat16, tag="s")
                    nc.vector.scalar_tensor_tensor(
                        out=s[:], in0=psA[:], scalar=dsb[:, m:m + 1], in1=bsb[:],
                        op0=mult, op1=add)
                    h = wp.tile([128, HID], dt.bfloat16, tag="h")
                    nc.scalar.activation(h[:], s[:], relu)
                    pt = pp2.tile([HID, 128], dt.bfloat16, tag="pt")
                    nc.tensor.transpose(pt[:], h[:], ident[:])
                    hT = wp.tile([HID, 128], dt.bfloat16, tag="hT")
                    nc.vector.tensor_copy(hT[:], pt[:])
                    if layer == 0:
                        ps2 = pp.tile([128, HID], dt.float32, tag="acc")
                        nc.tensor.matmul(ps2[:], hT[:], w2sb[:], start=True, stop=True)
                        hw2 = wp.tile([128, HID], dt.bfloat16, tag="hw2")
                        nc.vector.tensor_scalar_mul(hw2[:], ps2[:], dsb[:, m:m + 1])
                        nc.sync.dma_start(out=slice_d[1][mc, :], in_=hw2[:])
                    else:
                        pz = pp2.tile([HID2, 128], dt.float32, tag="pz")
                        nc.tensor.matmul(pz[:], wh1sb[:], hT[:], start=True, stop=True)
                        zb = wp.tile([HID2, 128], dt.bfloat16, tag="zb")
                        nc.scalar.activation(zb[:], pz[:], relu, bias=bh1sb[:])
                        po = pp2.tile([OUT, 128], dt.float32, tag="po")
                        nc.tensor.matmul(po[:], wh2sb[:], zb[:], start=True, stop=True)
                        ob = wp.tile([OUT, 128], dt.float32, tag="ob")
                        nc.vector.tensor_scalar_add(ob[:], po[:], bh2sb[:])
                        nc.sync.dma_start(out=outT[:, mc], in_=ob[:])

            agg_layer(0)
            agg_layer(1)

    nc.finalize()
    return nc


def kernel(x, edge_index, W1, b1, W2, b2, Wh1, bh1, Wh2, bh2, _trace=False):
    x = np.asarray(x, np.float32)
    src = np.asarray(edge_index[0], np.int64)
    dst = np.asarray(edge_index[1], np.int64)

    offs, dstv, n_m = _build_schedule(src, dst)
    sig = tuple(n_m)
    if sig not in _compiled:
        _compiled[sig] = _build_program(n_m)
    nc = _compiled[sig]

    deg = np.bincount(dst, minlength=N).astype(np.float64) + 1.0
    dinv = (1.0 / np.sqrt(deg)).astype(np.float32)

    W1 = np.asarray(W1, np.float32)
    w1p = np.concatenate([W1[:128], W1[128:]], axis=1)
    b1f = np.tile(np.asarray(b1, np.float32)[None, :], (128, 1))
    b2f = np.tile(np.asarray(b2, np.float32)[None, :], (128, 1))
    bh1c = np.asarray(bh1, np.float32)[:, None]
    bh2c = np.asarray(bh2, np.float32)[:, None]
    iota = np.tile(np.arange(128, dtype=np.float32)[None, :], (128, 1))
    ident = np.eye(128, dtype=np.float32)

    in_maps = []
    for c in range(8):
        xs = np.zeros((NS, IN_CH), np.float32)
        xs[:NS_RAW] = x[c * NS_RAW:(c + 1) * NS_RAW] * dinv[c * NS_RAW:(c + 1) * NS_RAW, None]
        dv = np.ones(NS, np.float32)
        dv[:NS_RAW] = dinv[c * NS_RAW:(c + 1) * NS_RAW]
        in_maps.append({
            "xT": np.ascontiguousarray(xs.T).astype(ml_dtypes.bfloat16),
            "w1p": w1p.astype(ml_dtypes.bfloat16),
            "w2": np.asarray(W2, np.float32).astype(ml_dtypes.bfloat16),
            "wh1": np.asarray(Wh1, np.float32).astype(ml_dtypes.bfloat16),
            "wh2": np.asarray(Wh2, np.float32).astype(ml_dtypes.bfloat16),
            "b1f": b1f, "b2f": b2f, "bh1": bh1c, "bh2": bh2c,
            "dinvP": np.ascontiguousarray(dv.reshape(NT, 128).T),
            "iotaP": iota.astype(ml_dtypes.bfloat16),
            "identP": ident.astype(ml_dtypes.bfloat16),
            "offsP": offs[c],
            "dstvP": dstv[c],
        })

    res = run_bass_kernel_spmd(nc, in_maps, list(range(8)), trace=_trace)
    out = np.empty((N, OUT), np.float32)
    for c in range(8):
        out[c * NS_RAW:(c + 1) * NS_RAW] = res.results[c]["outT"].T[:NS_RAW]
    if _trace:
        kernel.last_results = res
    return out
